# revision 1
# baseline (speedup 1.0000x reference)
"""DySimGCF message-passing kernel for 8 Trainium2 NeuronCores.

out[t, :] = sum_{e: to_e = t} norm_e * x[from_e, :]
norm_e = exp(a_e) / sqrt(Sin[to_e] * Sout[from_e])
Sin[t] = sum_{e: to_e = t} exp(a_e);  Sout[f] = sum_{e: from_e = f} exp(a_e)
(equivalent to the reference's max-stabilized segment softmaxes in exact
arithmetic; attrs are standard-normal so exp() cannot overflow in f32)

Distribution:
- Phase B: edges sharded by FROM-slice. Each core computes Sout for its
  12.5K nodes via dense windowed reductions (nodes grouped by exact
  out-degree), builds xp[f] = [x[f]/sqrt(Sout[f]) | 1 | 0pad] 128-col rows
  in a permuted "window" row order, AllGathers xp -> xp_full.
- Phase C: edges sharded by TO-slice. Per core, edges are bucketed into
  (128-target-block x source-chunk) cells of a fixed group count; per tile
  of TG 128-edge groups: dma_gather xp rows (512B each), batch-build
  one-hot weight matrices Q[e, g, t] = (t == tloc_e) * exp(a_e) in two DVE
  ops, then one fp32 matmul per group accumulates Q.T @ [X | 1] in PSUM
  (messages cols 0:64, Sin col 64). Cells drain into an SBUF accumulator;
  final per-block scale by 1/sqrt(Sin) writes the output slice densely.
"""

import numpy as np

import concourse.bacc as bacc
import concourse.bass as bass
import concourse.mybir as mybir
import concourse.tile as tile
from concourse.bass_utils import run_bass_kernel_spmd

# Problem constants (nn_DySimGCF_18202071400771)
N = 100000
D = 64
DP = 256  # padded bf16 xp row width (512B: [hi 64 | one | lo 64 | pad])
DR = 129  # used rhs cols: hi | one | lo starts at 65

C = 8  # cores
SL = N // C  # to/from slice per core = 12500
NB = -(-SL // 128)  # target blocks per core = 98
NCH = 4  # source chunks (2 cores each; 2*RPC rows must fit int16)
EPS = 1e-30
PAD_ATTR = -30.0

TRACE = False  # test.py may set kernel.TRACE = True
LAST_RESULT = None  # BassKernelResults of the last run (for test.py)

_PROGRAM_CACHE = {}


def _wrap16(idx):
    """[n] ints -> [128, n/16] int16 in the Q7 wrapped+replicated layout."""
    n = idx.shape[0]
    a = idx.reshape(n // 16, 16).T.astype(np.int16)
    return np.tile(a, (8, 1))


def _layout(edge_index, edge_attrs):
    """Host-side sharding/layout. Returns (meta, per-core input dict)."""
    f = edge_index[0].astype(np.int64)
    t = edge_index[1].astype(np.int64)
    a = edge_attrs.astype(np.float32)
    E = f.shape[0]
    nodes_core = np.arange(N) // SL

    # ---------------- Phase B structure ----------------
    deg = np.bincount(f, minlength=N)  # global out-degree
    DMAXB = int(deg.max())
    W = np.zeros((C, DMAXB + 1), np.int64)
    np.add.at(W, (nodes_core, deg), 1)
    W[:, 0] = 0
    Gd = np.ceil(W / 128).astype(np.int64).max(axis=0)  # groups per degree class
    ds = np.nonzero(Gd)[0]
    if int(Gd[ds].sum()) % 2 == 1:  # even total so the B gather splits evenly
        Gd[ds[0]] += 1
    Bq = np.zeros(DMAXB + 1, np.int64)  # group-column base per class
    FBq = np.zeros(DMAXB + 1, np.int64)  # free-col base per class
    gb = 0
    fb = 0
    for d in ds:
        Bq[d] = gb
        FBq[d] = fb
        gb += int(Gd[d])
        fb += int(Gd[d]) * int(d)
    GB = gb  # total window groups
    FBT = fb  # total attrB cols
    RPC = 128 * GB  # xp rows per core
    assert 2 * RPC <= 32768, (GB, RPC)

    # window assignment: per (core, degree) class, present nodes in node order
    pres = deg > 0
    order = np.lexsort((np.arange(N), deg, nodes_core))
    so = order[pres[order]]  # present nodes sorted by (core, deg, node)
    so_core = nodes_core[so]
    so_deg = deg[so]
    newg = np.ones(len(so), bool)
    newg[1:] = (so_core[1:] != so_core[:-1]) | (so_deg[1:] != so_deg[:-1])
    starts = np.flatnonzero(newg)
    lens = np.diff(np.append(starts, len(so)))
    rank = np.arange(len(so)) - np.repeat(starts, lens)
    w_gc = Bq[so_deg] + rank // 128
    w_p = rank % 128
    row_of = np.zeros(N, np.int64)
    row_of[so] = w_gc * 128 + w_p

    # attrB + gidxB
    oc = f // SL
    eo = np.argsort(f, kind="stable")
    ef = f[eo]
    node_start = np.zeros(N + 1, np.int64)
    node_start[1:] = np.cumsum(np.bincount(ef, minlength=N))
    j_in_node = np.arange(E) - node_start[ef]
    ed = deg[ef]
    e_gc = row_of[ef] // 128
    e_p = row_of[ef] % 128
    e_col = FBq[ed] + (e_gc - Bq[ed]) * ed + j_in_node
    attrB = np.full((C, 128, FBT), PAD_ATTR, np.float32)
    attrB[oc[eo], e_p, e_col] = a[eo]
    gidxB = np.zeros((C, RPC), np.int64)
    gidxB[so_core, w_gc * 128 + w_p] = so - so_core * SL
    gidxB_w = np.stack(
        [
            np.stack([_wrap16(gidxB[c, h * (RPC // 2) : (h + 1) * (RPC // 2)])
                      for h in range(2)])
            for c in range(C)
        ]
    )  # [C, 2, 128, RPC//32]

    # ---------------- Phase C structure ----------------
    tcore = t // SL
    tloc = t - tcore * SL
    blk = tloc // 128
    trel = (tloc % 128).astype(np.float32)
    k = oc // 2  # source chunk
    grow = oc * RPC + row_of[f]
    lidx = grow - k * (2 * RPC)  # chunk-local xp row

    cell = (tcore * NCH + k) * NB + blk
    cnt = np.bincount(cell, minlength=C * NCH * NB)
    GCELL = int(-(-cnt.max() // 128))
    CAP = 128 * GCELL
    ceo = np.argsort(cell, kind="stable")
    ccel = cell[ceo]
    cstart = np.zeros(C * NCH * NB + 1, np.int64)
    cstart[1:] = np.cumsum(np.bincount(ccel, minlength=C * NCH * NB))
    r_in_cell = np.arange(E) - cstart[ccel]
    cell_local = ccel % (NCH * NB)  # per-core cell id
    slot = cell_local * CAP + r_in_cell
    SLOTS = NCH * NB * CAP
    COLS = SLOTS // 128
    attrC = np.full((C, 128, COLS), PAD_ATTR, np.float32)
    tlocC = np.full((C, 128, COLS), -1.0, np.float32)
    lidxC = np.zeros((C, 128, COLS), np.int64)
    sc = ccel // (NCH * NB)
    attrC[sc, slot % 128, slot // 128] = a[ceo]
    tlocC[sc, slot % 128, slot // 128] = trel[ceo]
    lidxC[sc, slot % 128, slot // 128] = lidx[ceo]

    # gather tiles: TG groups each, within one chunk
    GPC = NB * GCELL  # groups per chunk
    TG = 1
    for cand in range(32, 0, -1):
        if GPC % cand == 0:
            TG = cand
            break
    TPC = GPC // TG  # tiles per chunk
    NT = NCH * TPC
    lidx_flat = lidxC.reshape(C, 128, NT, TG).transpose(0, 2, 3, 1).reshape(C, NT, TG * 128)
    gidxC_w = np.zeros((C, NT, 128, TG * 8), np.int16)
    for c in range(C):
        for ti in range(NT):
            gidxC_w[c, ti] = _wrap16(lidx_flat[c, ti])

    iota = np.tile(np.arange(128, dtype=np.float32), (128, 1))

    meta = dict(
        DMAXB=DMAXB, Gd=tuple(int(g) for g in Gd), ds=tuple(int(d) for d in ds),
        Bq=tuple(int(x) for x in Bq), FBq=tuple(int(x) for x in FBq),
        GB=GB, FBT=FBT, RPC=RPC, GCELL=GCELL, CAP=CAP, SLOTS=SLOTS,
        COLS=COLS, TG=TG, TPC=TPC, NT=NT,
    )
    in_maps = []
    for c in range(C):
        in_maps.append({
            "x_slice": None,  # filled by caller
            "attrB": attrB[c],
            "gidxB": gidxB_w[c],
            "attrC": attrC[c],
            "tlocC": tlocC[c],
            "gidxC": gidxC_w[c],
            "iota": iota,
        })
    return meta, in_maps


def _build_program(meta):
    GB = meta["GB"]
    FBT = meta["FBT"]
    RPC = meta["RPC"]
    GCELL = meta["GCELL"]
    COLS = meta["COLS"]
    TG = meta["TG"]
    TPC = meta["TPC"]
    NT = meta["NT"]
    ds = meta["ds"]
    Gd = meta["Gd"]
    Bq = meta["Bq"]
    FBq = meta["FBq"]

    nc = bacc.Bacc("TRN2", target_bir_lowering=False, debug=False,
                   num_devices=C, num_swdge_queues=4)

    x_slice = nc.dram_tensor("x_slice", [SL, D], mybir.dt.float32, kind="ExternalInput")
    attrB = nc.dram_tensor("attrB", [128, FBT], mybir.dt.float32, kind="ExternalInput")
    gidxB = nc.dram_tensor("gidxB", [2, 128, RPC // 32], mybir.dt.int16, kind="ExternalInput")
    attrC = nc.dram_tensor("attrC", [128, COLS], mybir.dt.float32, kind="ExternalInput")
    tlocC = nc.dram_tensor("tlocC", [128, COLS], mybir.dt.float32, kind="ExternalInput")
    gidxC = nc.dram_tensor("gidxC", [NT, 128, TG * 8], mybir.dt.int16, kind="ExternalInput")
    iota_d = nc.dram_tensor("iota", [128, 128], mybir.dt.float32, kind="ExternalInput")
    out = nc.dram_tensor("out", [SL, D], mybir.dt.float32, kind="ExternalOutput")

    xpc = nc.dram_tensor("xpc", [RPC, DP], mybir.dt.bfloat16)
    xp_full = nc.dram_tensor("xp_full", [C * RPC, DP], mybir.dt.bfloat16, addr_space="Shared")

    with tile.TileContext(nc) as tc:
        with tc.tile_pool(name="cst", bufs=1) as cst:
            iota_t = cst.tile([128, 128], mybir.dt.float32)
            nc.sync.dma_start(iota_t[:], iota_d.ap())
            eps_t = cst.tile([128, 1], mybir.dt.float32)
            nc.vector.memset(eps_t[:], EPS)

            # ---------------- Phase B ----------------
            with tc.tile_pool(name="bph", bufs=1) as bph:
                attrB_t = bph.tile([128, FBT], mybir.dt.float32)
                nc.sync.dma_start(attrB_t[:], attrB.ap())
                expB = bph.tile([128, FBT], mybir.dt.float32)
                nc.scalar.activation(expB[:], attrB_t[:],
                                     mybir.ActivationFunctionType.Exp)
                sout = bph.tile([128, GB], mybir.dt.float32)
                for d in ds:
                    g = Gd[d]
                    seg = expB[:, FBq[d] : FBq[d] + g * d].rearrange(
                        "p (g d) -> p g d", d=d)
                    nc.vector.tensor_reduce(
                        sout[:, Bq[d] : Bq[d] + g], seg,
                        axis=mybir.AxisListType.X, op=mybir.AluOpType.add)
                stdB = bph.tile([128, GB], mybir.dt.float32)
                nc.scalar.activation(stdB[:], sout[:],
                                     mybir.ActivationFunctionType.Sqrt, bias=eps_t[:])
                rB = bph.tile([128, GB], mybir.dt.float32)
                nc.vector.reciprocal(rB[:], stdB[:])

                H = GB // 2
                xB = bph.tile([128, GB, D], mybir.dt.float32)
                for h in range(2):
                    gi_t = bph.tile([128, RPC // 32], mybir.dt.int16, tag="gib")
                    nc.sync.dma_start(gi_t[:], gidxB.ap()[h])
                    nc.gpsimd.dma_gather(
                        out_ap=xB[:, h * H : (h + 1) * H, :],
                        in_ap=x_slice.ap(),
                        idxs_ap=gi_t[:],
                        num_idxs=H * 128, num_idxs_reg=H * 128,
                        elem_size=D, single_packet=False, queue_num=h)
                x1 = bph.tile([128, GB, D], mybir.dt.float32)
                nc.vector.tensor_tensor(
                    x1[:], xB[:],
                    rB[:].unsqueeze(-1).broadcast_to([128, GB, D]),
                    mybir.AluOpType.mult)
                xps = bph.tile([128, GB, DP], mybir.dt.bfloat16)
                nc.vector.memset(xps[:, :, 2 * D + 1 :], 0.0)
                nc.vector.memset(xps[:, :, D : D + 1], 1.0)
                nc.vector.tensor_copy(xps[:, :, 0:D], x1[:])  # hi = bf16(x')
                nc.vector.tensor_tensor(  # lo = bf16(x' - hi)
                    xps[:, :, D + 1 : 2 * D + 1], x1[:], xps[:, :, 0:D],
                    mybir.AluOpType.subtract)
                nc.sync.dma_start(xpc.ap().rearrange("(g p) c -> p g c", p=128),
                                  xps[:])
                nc.gpsimd.collective_compute(
                    "AllGather", mybir.AluOpType.bypass,
                    replica_groups=[list(range(C))],
                    ins=[xpc.ap()], outs=[xp_full.ap()])

            # ---------------- Phase C ----------------
            with (
                tc.tile_pool(name="xg", bufs=2) as xgp,
                tc.tile_pool(name="qg", bufs=2) as qgp,
                tc.tile_pool(name="meta_p", bufs=3) as mp,
                tc.tile_pool(name="accp", bufs=1) as accp,
                tc.tile_pool(name="psp", bufs=4, space="PSUM") as psp,
            ):
                acc = accp.tile([128, NB, D + 1], mybir.dt.float32)
                nc.vector.memset(acc[:], 0.0)

                ps = None
                for ti in range(NT):
                    k = ti // TPC
                    gi = mp.tile([128, TG * 8], mybir.dt.int16, tag="gic")
                    nc.sync.dma_start(gi[:], gidxC.ap()[ti])
                    at = mp.tile([128, TG], mybir.dt.float32, tag="atc")
                    nc.sync.dma_start(at[:], attrC.ap()[:, ti * TG : (ti + 1) * TG])
                    tl = mp.tile([128, TG], mybir.dt.float32, tag="tlc")
                    nc.sync.dma_start(tl[:], tlocC.ap()[:, ti * TG : (ti + 1) * TG])
                    ex = mp.tile([128, TG], mybir.dt.float32, tag="exc")
                    nc.scalar.activation(ex[:], at[:],
                                         mybir.ActivationFunctionType.Exp)
                    X = xgp.tile([128, TG, DP], mybir.dt.bfloat16, tag="X")
                    nc.gpsimd.dma_gather(
                        out_ap=X[:],
                        in_ap=xp_full.ap()[k * 2 * RPC : (k + 1) * 2 * RPC],
                        idxs_ap=gi[:],
                        num_idxs=TG * 128, num_idxs_reg=TG * 128,
                        elem_size=DP, single_packet=False, queue_num=ti % 4)
                    # expa hi/lo split (per-edge scalars)
                    exhi = mp.tile([128, TG], mybir.dt.bfloat16, tag="exhi")
                    nc.vector.tensor_copy(exhi[:], ex[:])
                    exlo = mp.tile([128, TG], mybir.dt.bfloat16, tag="exlo")
                    nc.vector.tensor_tensor(exlo[:], ex[:], exhi[:],
                                            mybir.AluOpType.subtract)
                    # batched one-hot build: M = (iota_t==tloc); Q{hi,lo} = M*expa{hi,lo}
                    M = qgp.tile([128, TG, 128], mybir.dt.bfloat16, tag="M")
                    nc.vector.tensor_tensor(
                        M[:],
                        iota_t[:].unsqueeze(1).broadcast_to([128, TG, 128]),
                        tl[:].unsqueeze(-1).broadcast_to([128, TG, 128]),
                        mybir.AluOpType.is_equal)
                    Qhi = qgp.tile([128, TG, 128], mybir.dt.bfloat16, tag="Qhi")
                    nc.vector.tensor_tensor(
                        Qhi[:], M[:],
                        exhi[:].unsqueeze(-1).broadcast_to([128, TG, 128]),
                        mybir.AluOpType.mult)
                    Qlo = qgp.tile([128, TG, 128], mybir.dt.bfloat16, tag="Qlo")
                    nc.vector.tensor_tensor(
                        Qlo[:], M[:],
                        exlo[:].unsqueeze(-1).broadcast_to([128, TG, 128]),
                        mybir.AluOpType.mult)
                    for gg in range(TG):
                        g_glob = ti * TG + gg
                        pos = g_glob % GCELL
                        if pos == 0:
                            ps = psp.tile([128, DR], mybir.dt.float32, tag="ps")
                        nc.tensor.matmul(out=ps[:], lhsT=Qhi[:, gg, :],
                                         rhs=X[:, gg, 0:DR],
                                         start=(pos == 0), stop=False)
                        nc.tensor.matmul(out=ps[:], lhsT=Qlo[:, gg, :],
                                         rhs=X[:, gg, 0:DR],
                                         start=False, stop=(pos == GCELL - 1))
                        if pos == GCELL - 1:
                            b = (g_glob // GCELL) % NB
                            nc.vector.tensor_add(acc[:, b, :], acc[:, b, :],
                                                 ps[:, 0 : D + 1])
                            nc.vector.tensor_add(acc[:, b, 0:D], acc[:, b, 0:D],
                                                 ps[:, D + 1 : 2 * D + 1])

                # final scale + output
                for b in range(NB):
                    rows = min(128, SL - b * 128)
                    stdc = mp.tile([128, 1], mybir.dt.float32, tag="stdc")
                    nc.scalar.activation(stdc[:], acc[:, b, D : D + 1],
                                         mybir.ActivationFunctionType.Sqrt,
                                         bias=eps_t[:])
                    rc = mp.tile([128, 1], mybir.dt.float32, tag="rc")
                    nc.vector.reciprocal(rc[:], stdc[:])
                    ot = mp.tile([128, D], mybir.dt.float32, tag="ot")
                    nc.scalar.activation(ot[:], acc[:, b, 0:D],
                                         mybir.ActivationFunctionType.Copy,
                                         scale=rc[:])
                    nc.sync.dma_start(out.ap()[b * 128 : b * 128 + rows], ot[:rows])

    nc.compile()
    return nc


def kernel(x, edge_index, edge_attrs):
    global LAST_RESULT
    meta, in_maps = _layout(edge_index, edge_attrs)
    key = tuple(sorted(meta.items()))
    if key not in _PROGRAM_CACHE:
        _PROGRAM_CACHE[key] = _build_program(meta)
    nc = _PROGRAM_CACHE[key]
    xf = np.ascontiguousarray(x, dtype=np.float32)
    for c in range(C):
        in_maps[c]["x_slice"] = xf[c * SL : (c + 1) * SL]
    res = run_bass_kernel_spmd(nc, in_maps, core_ids=list(range(C)), trace=TRACE)
    LAST_RESULT = res
    return np.concatenate([res.results[c]["out"] for c in range(C)], axis=0)



# revision 9
# speedup vs baseline: 1.7281x; 1.7281x over previous
"""DySimGCF message-passing kernel for 8 Trainium2 NeuronCores.

out[t, :] = sum_{e: to_e = t} norm_e * x[from_e, :]
norm_e = exp(a_e) / sqrt(Sin[to_e] * Sout[from_e])
Sin[t] = sum_{e: to_e = t} exp(a_e);  Sout[f] = sum_{e: from_e = f} exp(a_e)
(equivalent to the reference's max-stabilized segment softmaxes in exact
arithmetic; attrs are standard-normal so exp() cannot overflow in fp16)

Distribution (edge parallelism, target-sharded):
- Phase B: each core computes Sout for its 12.5K nodes via a dense
  max-degree-padded attr layout (no gather), builds xp[f] = fp16
  [x[f]/sqrt(Sout[f]) | 1 | 0pad] 128-col rows in NODE order, and
  AllGathers xp in two halves so the second collective overlaps Phase C.
- Phase C: edges sharded by TO-slice, bucketed into (128-target-block x
  source-chunk) cells.  Static structure per 8-block super-tile x chunk:
  3 full 128-edge groups per cell plus a small shared tail region for
  cell overflow (tail slots are packed, padded with negative gather
  indices at the end, and trimmed per-core via num_idxs_reg).  One fp16
  matmul per group accumulates Q.T @ [X | 1] into one PSUM bank per
  block, held open across all 4 chunks (2 passes); the final per-block
  scale by 1/sqrt(Sin) writes the output slice densely.
"""

import numpy as np

import concourse.bacc as bacc
import concourse.bass as bass
import concourse.mybir as mybir
import concourse.tile as tile
from concourse.bass_utils import run_bass_kernel_spmd

# Problem constants (nn_DySimGCF_18202071400771)
N = 100000
D = 64
DP = 128      # fp16 xp row width (256B: [x' 64 | one | 0pad 63])
DR = D + 1    # used rhs cols

C = 8         # cores
SL = N // C   # nodes per core = 12500
NB = -(-SL // 128)      # target blocks per core = 98
RPCN = NB * 128         # padded xp rows per core = 12544
HB = RPCN // 2          # half-buffer rows per core = 6272
CHROWS = 4 * HB         # rows per source chunk = 25088 (< 32768, int16-safe)
D3 = 3                  # static full groups per cell
SZS = [8] * (NB // 8) + ([NB % 8] if NB % 8 else [])  # super-tile sizes
NSTC = len(SZS)
EPS = 1e-20
EPS_B = 1e-6  # Phase-B Sout bias: keeps 1/sqrt finite in fp16 for deg-0 nodes
PAD_ATTR = -1000.0
PAD_TL = -512.0

TRACE = False   # test.py may set kernel.TRACE = True
LAST_RESULT = None  # BassKernelResults of the last run (for test.py)

_PROGRAM_CACHE = {}


def _wrap16(idx):
    """[n] ints -> [128, n/16] int16 in the wrapped+replicated idx layout."""
    n = idx.shape[0]
    a = idx.reshape(n // 16, 16).T.astype(np.int16)
    return np.tile(a, (8, 1))


def _layout(edge_index, edge_attrs):
    """Host-side sharding/layout. Returns (meta, per-core input dict)."""
    f = edge_index[0].astype(np.int64)
    t = edge_index[1].astype(np.int64)
    a = edge_attrs.astype(np.float32)
    E = f.shape[0]

    # ---------------- Phase B structure (dense, node order) ----------------
    deg = np.bincount(f, minlength=N)
    DMAXB = int(deg.max())
    oc = f // SL
    eo = np.argsort(f, kind="stable")
    ef = f[eo]
    node_start = np.zeros(N + 1, np.int64)
    node_start[1:] = np.cumsum(np.bincount(ef, minlength=N))
    j_in_node = np.arange(E) - node_start[ef]
    floc = ef - (ef // SL) * SL
    attrB = np.full((C, 128, NB * DMAXB), PAD_ATTR, np.float32)
    attrB[ef // SL, floc % 128, (floc // 128) * DMAXB + j_in_node] = a[eo]

    # ---------------- Phase C structure ----------------
    c_src = f // SL
    l_src = f - c_src * SL
    hh = l_src // HB
    k = 2 * hh + (c_src >= 4)                 # source chunk 0..3
    lidx = (c_src % 4) * HB + (l_src - hh * HB)  # chunk-local xp row

    tcore = t // SL
    tloc = t - tcore * SL
    blk = tloc // 128
    trel = tloc % 128
    s_of = blk // 8                            # super-tile 0..NSTC-1
    brel = blk - 8 * s_of
    P = k // 2
    qi = k % 2
    ti = P * (2 * NSTC) + s_of * 2 + qi        # tile id 0..(4*NSTC-1)
    NT = 4 * NSTC

    cell = (tcore * 4 + k) * NB + blk
    ceo = np.argsort(cell, kind="stable")
    ccel = cell[ceo]
    cstart = np.zeros(C * 4 * NB + 1, np.int64)
    cstart[1:] = np.cumsum(np.bincount(ccel, minlength=C * 4 * NB))
    r_in_cell = np.arange(E) - cstart[ccel]

    tco = tcore[ceo]
    tio = ti[ceo]
    bro = brel[ceo]
    so = s_of[ceo]
    FULLCAP = D3 * 128
    is_full = r_in_cell < FULLCAP

    # tail ranks: excess edges ranked within (tcore, ti) sorted by (brel, r)
    exm = ~is_full
    exc_key = (tco[exm] * NT + tio[exm]) * (NB * 1024) + bro[exm] * 1024 + (
        r_in_cell[exm] - FULLCAP)
    exo = np.argsort(exc_key, kind="stable")
    ztile = tco[exm][exo] * NT + tio[exm][exo]
    tstart = np.zeros(C * NT + 1, np.int64)
    tstart[1:] = np.cumsum(np.bincount(ztile, minlength=C * NT))
    trank_sorted = np.arange(exm.sum()) - tstart[ztile]
    trank = np.empty(exm.sum(), np.int64)
    trank[exo] = trank_sorted
    tail_cnt = np.bincount(ztile, minlength=C * NT).reshape(C, NT)

    # per-super-tile tail capacity (shared across cores/chunks: static program)
    TCs = []
    for s in range(NSTC):
        cols = [P0 * (2 * NSTC) + s * 2 + q for P0 in range(2) for q in range(2)]
        m = int(tail_cnt[:, cols].max())
        TCs.append(max(1, -(-m // 128)))
    TCs = tuple(TCs)

    NGs = tuple(SZS[s] * D3 + TCs[s] for s in range(NSTC))
    gcb = np.zeros(NT + 1, np.int64)   # group col base per tile
    for t_i in range(NT):
        s = (t_i % (2 * NSTC)) // 2
        gcb[t_i + 1] = gcb[t_i] + NGs[s]
    GTOT = int(gcb[NT])

    # slot assignment
    slot_col = np.empty(E, np.int64)   # group column (global)
    slot_p = np.empty(E, np.int64)     # partition
    tlv = np.empty(E, np.float32)      # tloc encoding
    fm = is_full
    slot_col[fm] = gcb[tio[fm]] + bro[fm] * D3 + r_in_cell[fm] // 128
    slot_p[fm] = r_in_cell[fm] % 128
    tlv[fm] = trel[ceo][fm]
    szD3 = np.array([SZS[s] * D3 for s in range(NSTC)], np.int64)
    slot_col[exm] = gcb[tio[exm]] + szD3[so[exm]] + trank // 128
    slot_p[exm] = trank % 128
    tlv[exm] = bro[exm] * 128 + trel[ceo][exm]

    attrC = np.full((C, 128, GTOT), PAD_ATTR, np.float32)
    tlocC = np.full((C, 128, GTOT), PAD_TL, np.float16)
    idxF = np.zeros((C, GTOT * 128), np.int64)  # full-region pad -> row 0
    # tail region pads -> -1 (trailing, trimmed by num_idxs_reg/kernel)
    for t_i in range(NT):
        s = (t_i % (2 * NSTC)) // 2
        b0 = gcb[t_i] + SZS[s] * D3
        idxF[:, b0 * 128 : gcb[t_i + 1] * 128] = -1
    attrC[tco, slot_p, slot_col] = a[ceo]
    tlocC[tco, slot_p, slot_col] = tlv.astype(np.float16)
    idxF[tco, slot_col * 128 + slot_p] = lidx[ceo]

    gidxC = np.zeros((C, 128, GTOT * 8), np.int16)
    for cc in range(C):
        for t_i in range(NT):
            sl_ = slice(int(gcb[t_i]) * 128, int(gcb[t_i + 1]) * 128)
            gidxC[cc, :, gcb[t_i] * 8 : gcb[t_i + 1] * 8] = _wrap16(idxF[cc, sl_])

    cnts = np.zeros((C, NT), np.int32)
    for t_i in range(NT):
        s = (t_i % (2 * NSTC)) // 2
        cnts[:, t_i] = SZS[s] * D3 * 128 + tail_cnt[:, t_i]

    iota = np.tile(np.arange(1024, dtype=np.float16), (128, 1))

    meta = dict(DMAXB=DMAXB, TCs=TCs, GTOT=GTOT,
                gcb=tuple(int(x) for x in gcb))
    in_maps = []
    for cc in range(C):
        in_maps.append({
            "x_slice": None,  # filled by caller
            "attrB": attrB[cc],
            "attrC": attrC[cc],
            "tlocC": tlocC[cc],
            "gidxC": gidxC[cc],
            "cnts": cnts[cc : cc + 1],
            "iota": iota,
        })
    return meta, in_maps


def _build_program(meta):
    DMAXB = meta["DMAXB"]
    TCs = meta["TCs"]
    GTOT = meta["GTOT"]
    gcb = meta["gcb"]
    NT = 4 * NSTC
    NGMAX = max(SZS[s] * D3 + TCs[s] for s in range(NSTC))
    TCMAX = max(TCs)

    f16 = mybir.dt.float16
    f32 = mybir.dt.float32

    nc = bacc.Bacc("TRN2", target_bir_lowering=False, debug=False,
                   num_devices=C, num_swdge_queues=4)

    x_slice = nc.dram_tensor("x_slice", [RPCN, D], f32, kind="ExternalInput")
    attrB = nc.dram_tensor("attrB", [128, NB * DMAXB], f32, kind="ExternalInput")
    attrC = nc.dram_tensor("attrC", [128, GTOT], f32, kind="ExternalInput")
    tlocC = nc.dram_tensor("tlocC", [128, GTOT], f16, kind="ExternalInput")
    gidxC = nc.dram_tensor("gidxC", [128, GTOT * 8], mybir.dt.int16,
                           kind="ExternalInput")
    cnts = nc.dram_tensor("cnts", [1, NT], mybir.dt.int32, kind="ExternalInput")
    iota_d = nc.dram_tensor("iota", [128, 1024], f16, kind="ExternalInput")
    out = nc.dram_tensor("out", [SL, D], f32, kind="ExternalOutput")

    xpc = nc.dram_tensor("xpc", [RPCN, DP], f16)
    xp_half = [
        nc.dram_tensor(f"xp_{h}", [C * HB, DP], f16, addr_space="Shared")
        for h in range(2)
    ]

    nreg = [nc.alloc_register(mybir.EngineType.Pool, f"cnt{i}") for i in range(4)]

    with tile.TileContext(nc) as tc:
        with tc.tile_pool(name="cst", bufs=1) as cst:
            iota_t = cst.tile([128, 1024], f16)
            nc.sync.dma_start(iota_t[:], iota_d.ap())
            eps_t = cst.tile([128, 1], f32)
            nc.vector.memset(eps_t[:], EPS)
            epsb_t = cst.tile([128, 1], f32)
            nc.vector.memset(epsb_t[:], EPS_B)
            cnts_t = cst.tile([1, NT], mybir.dt.int32)
            nc.sync.dma_start(cnts_t[:], cnts.ap())

            # ---------------- Phase B ----------------
            with tc.tile_pool(name="bph", bufs=1) as bph:
                attrB_t = bph.tile([128, NB, DMAXB], f32)
                nc.sync.dma_start(
                    attrB_t[:],
                    attrB.ap().rearrange("p (b j) -> p b j", j=DMAXB))
                expB = bph.tile([128, NB, DMAXB], f32)
                nc.scalar.activation(expB[:], attrB_t[:],
                                     mybir.ActivationFunctionType.Exp)
                sout = bph.tile([128, NB], f32)
                nc.vector.tensor_reduce(sout[:], expB[:],
                                        axis=mybir.AxisListType.X,
                                        op=mybir.AluOpType.add)
                stdB = bph.tile([128, NB], f32)
                nc.scalar.activation(stdB[:], sout[:],
                                     mybir.ActivationFunctionType.Sqrt,
                                     bias=epsb_t[:])
                rB = bph.tile([128, NB], f32)
                nc.vector.reciprocal(rB[:], stdB[:])

                xd = bph.tile([128, NB, D], f32)
                nc.sync.dma_start(
                    xd[:], x_slice.ap().rearrange("(g p) c -> p g c", p=128))
                xps = bph.tile([128, NB, DP], f16)
                nc.vector.memset(xps[:, :, D + 1 :], 0.0)
                nc.vector.memset(xps[:, :, D : D + 1], 1.0)
                nc.vector.tensor_tensor(
                    xps[:, :, 0:D], xd[:],
                    rB[:].unsqueeze(-1).broadcast_to([128, NB, D]),
                    mybir.AluOpType.mult)
                for h in range(2):
                    nc.sync.dma_start(
                        xpc.ap()[h * HB : (h + 1) * HB].rearrange(
                            "(g p) c -> p g c", p=128),
                        xps[:, h * (NB // 2) : (h + 1) * (NB // 2), :])
                    nc.gpsimd.collective_compute(
                        "AllGather", mybir.AluOpType.bypass,
                        replica_groups=[list(range(C))],
                        ins=[xpc.ap()[h * HB : (h + 1) * HB]],
                        outs=[xp_half[h].ap()])

            # ---------------- Phase C ----------------
            with (
                tc.tile_pool(name="xg", bufs=3) as xgp,
                tc.tile_pool(name="qg", bufs=2) as qgp,
                tc.tile_pool(name="meta_p", bufs=4) as mp,
                tc.tile_pool(name="accp", bufs=1) as accp,
                tc.tile_pool(name="psp", bufs=1, space="PSUM") as psp,
            ):
                accA = accp.tile([128, NB, DR], f32)

                for P in range(2):
                    xp_buf = xp_half[P]
                    for s in range(NSTC):
                        sz = SZS[s]
                        TC = TCs[s]
                        NG = sz * D3 + TC
                        ps = [psp.tile([128, DR], f32, tag=f"ps{b}",
                                       name=f"ps{b}")
                              for b in range(sz)]
                        for qi in range(2):
                            t_i = P * (2 * NSTC) + s * 2 + qi
                            gb = gcb[t_i]
                            nf = sz * D3
                            gi = mp.tile([128, NGMAX * 8], mybir.dt.int16,
                                         tag="gic")
                            nc.sync.dma_start(
                                gi[:, : NG * 8],
                                gidxC.ap()[:, gb * 8 : (gb + NG) * 8])
                            at = mp.tile([128, NGMAX], f32, tag="atc")
                            nc.sync.dma_start(at[:, :NG],
                                              attrC.ap()[:, gb : gb + NG])
                            tl = mp.tile([128, NGMAX], f16, tag="tlc")
                            nc.sync.dma_start(tl[:, :NG],
                                              tlocC.ap()[:, gb : gb + NG])
                            ex = mp.tile([128, NGMAX], f16, tag="exc")
                            nc.scalar.activation(
                                ex[:, :NG], at[:, :NG],
                                mybir.ActivationFunctionType.Exp)
                            rg = nreg[t_i % 4]
                            nc.gpsimd.reg_load(rg, cnts_t[0:1, t_i : t_i + 1])
                            X = xgp.tile([128, NGMAX, DP], f16, tag="X")
                            # trimmed tail slots are never gathered; zero them
                            # so Q=0 x X stays 0 (garbage could be NaN/inf)
                            nc.vector.memset(X[:, nf:NG, :], 0.0)
                            nc.gpsimd.dma_gather(
                                out_ap=X[:, :NG, :],
                                in_ap=xp_buf.ap()[qi * CHROWS : (qi + 1) * CHROWS],
                                idxs_ap=gi[:, : NG * 8],
                                num_idxs=NG * 128, num_idxs_reg=rg,
                                elem_size=DP, single_packet=False,
                                queue_num=t_i % 4)
                            # Q builds: full groups then shared tails
                            Qf = qgp.tile([128, SZS[0] * D3, 128], f16, tag="Qf")
                            nc.vector.tensor_tensor(
                                Qf[:, :nf, :],
                                iota_t[:, 0:128].unsqueeze(1).broadcast_to(
                                    [128, nf, 128]),
                                tl[:, :nf].unsqueeze(-1).broadcast_to(
                                    [128, nf, 128]),
                                mybir.AluOpType.is_equal)
                            nc.vector.tensor_tensor(
                                Qf[:, :nf, :], Qf[:, :nf, :],
                                ex[:, :nf].unsqueeze(-1).broadcast_to(
                                    [128, nf, 128]),
                                mybir.AluOpType.mult)
                            Qt = qgp.tile([128, TCMAX, SZS[0] * 128], f16,
                                          tag="Qt")
                            nc.vector.tensor_tensor(
                                Qt[:, :TC, : sz * 128],
                                iota_t[:, 0 : sz * 128].unsqueeze(1).broadcast_to(
                                    [128, TC, sz * 128]),
                                tl[:, nf : nf + TC].unsqueeze(-1).broadcast_to(
                                    [128, TC, sz * 128]),
                                mybir.AluOpType.is_equal)
                            nc.vector.tensor_tensor(
                                Qt[:, :TC, : sz * 128], Qt[:, :TC, : sz * 128],
                                ex[:, nf : nf + TC].unsqueeze(-1).broadcast_to(
                                    [128, TC, sz * 128]),
                                mybir.AluOpType.mult)
                            for b in range(sz):
                                for dd in range(D3):
                                    nc.tensor.matmul(
                                        out=ps[b][:],
                                        lhsT=Qf[:, b * D3 + dd, :],
                                        rhs=X[:, b * D3 + dd, 0:DR],
                                        start=(qi == 0 and dd == 0),
                                        stop=False)
                            for tcg in range(TC):
                                for b in range(sz):
                                    nc.tensor.matmul(
                                        out=ps[b][:],
                                        lhsT=Qt[:, tcg, b * 128 : (b + 1) * 128],
                                        rhs=X[:, nf + tcg, 0:DR],
                                        start=False,
                                        stop=(qi == 1 and tcg == TC - 1))
                        # drain super-tile
                        for b in range(sz):
                            blkid = s * 8 + b
                            if P == 0:
                                nc.vector.tensor_copy(accA[:, blkid, :],
                                                      ps[b][:])
                            else:
                                t65 = mp.tile([128, DR], f32, tag="t65")
                                nc.vector.tensor_add(t65[:], ps[b][:],
                                                     accA[:, blkid, :])
                                stdc = mp.tile([128, 1], f32, tag="stdc")
                                nc.scalar.activation(
                                    stdc[:], t65[:, D : D + 1],
                                    mybir.ActivationFunctionType.Sqrt,
                                    bias=eps_t[:])
                                rc = mp.tile([128, 1], f32, tag="rc")
                                nc.vector.reciprocal(rc[:], stdc[:])
                                ot = mp.tile([128, D], f32, tag="ot")
                                nc.scalar.activation(
                                    ot[:], t65[:, 0:D],
                                    mybir.ActivationFunctionType.Copy,
                                    scale=rc[:])
                                rows = min(128, SL - blkid * 128)
                                nc.sync.dma_start(
                                    out.ap()[blkid * 128 : blkid * 128 + rows],
                                    ot[:rows])

    nc.compile()
    return nc


def kernel(x, edge_index, edge_attrs):
    global LAST_RESULT
    meta, in_maps = _layout(edge_index, edge_attrs)
    key = tuple(sorted((k, v) for k, v in meta.items() if k != "gcb")) + (
        meta["gcb"],)
    if key not in _PROGRAM_CACHE:
        _PROGRAM_CACHE[key] = _build_program(meta)
    nc = _PROGRAM_CACHE[key]
    xf = np.zeros((C, RPCN, D), np.float32)
    xs = np.ascontiguousarray(x, dtype=np.float32).reshape(C, SL, D)
    xf[:, :SL, :] = xs
    for cc in range(C):
        in_maps[cc]["x_slice"] = xf[cc]
    res = run_bass_kernel_spmd(nc, in_maps, core_ids=list(range(C)), trace=TRACE)
    LAST_RESULT = res
    return np.concatenate([res.results[cc]["out"] for cc in range(C)], axis=0)
